# revision 1
# baseline (speedup 1.0000x reference)
"""CenterCut2 Trainium2 kernel.

For each sample b: find argmax of power = sum_c x[b,c]^2 over the (D,H,W)
volume, then extract the 16x32x32 window centered on the peak with circular
wraparound (equivalent to reference's per-sample roll + center crop).

Sharding: pure data parallelism, 4 samples per core across 8 cores.

Per-core device program (samples s=0..3, volumes v=2s+c laid out [128, 8192]
with flat voxel index = p*8192 + f = dd*16384 + hh*128 + w):
  Per sample (pipelined so sample s's window extraction overlaps sample s+1's
  streaming):
  1. Stream both channel volumes; power = x0^2 (ACT) + x1^2 (ACT, in place)
     summed on DVE.
  2. vector.max + max_index give the per-partition argmax; partition_all_reduce
     max with a BIG-constant tie-break selects the global flat index (lowest
     flat index on exact ties, matching jnp.argmax); DVE integer ops decompose
     it into d, h0, w0, s_h and the two 32-row h-chunk ids c0/c1.
  3. One 64-descriptor dma_gather (16KB per descriptor) pulls, for each of the
     16 d-slices and 2 channels, the two 32-row h-chunks covering the
     h-window; rows land at partitions chunk*32 + c*16 + i (base 0).
  4. The two chunk halves are merged into a [32, 64, 160] tile doubled along w
     (merge copies split between ACT and DVE; partition-shifted halves on DVE),
     and a single register-offset (bass.ds) strided copy extracts the
     [32h x 32w] window. One [32, 1024] DMA writes the sample's output.
"""
import sys

sys.path.insert(0, "/opt/trn_rl_repo")

import numpy as np

import concourse.bass as bass
import concourse.bacc as bacc
import concourse.mybir as mybir
from concourse.tile import TileContext
from concourse.tile_rust import add_dep_helper
from concourse.bass_utils import run_bass_kernel_spmd
from concourse.bass_isa import ReduceOp

F32 = mybir.dt.float32
I32 = mybir.dt.int32
I16 = mybir.dt.int16
A = mybir.AluOpType
DVE = mybir.EngineType.DVE

N_CORES = 8
S_PER_CORE = 4          # samples per core
N_VOLS = 2 * S_PER_CORE # channel volumes per core
VOL = 64 * 128 * 128    # voxels per volume
FREE = VOL // 128       # 8192 free elements per partition
CHUNK = 4096            # streaming chunk (2 MiB per DMA)
BIG = float(1 << 21)

_cache = {}


def _build(loop_k=None):
    nc = bacc.Bacc("TRN2", target_bir_lowering=False, debug=False, num_devices=N_CORES)
    x = nc.dram_tensor("x", [N_VOLS, 128, FREE], F32, kind="ExternalInput")
    y = nc.dram_tensor("y", [128, 1024], F32, kind="ExternalOutput")

    iota_base_c = nc.inline_tensor(
        (np.arange(128, dtype=np.float32) * FREE).reshape(128, 1), name="iota_base"
    )
    iotaq_c = nc.inline_tensor(np.arange(16, dtype=np.int32).reshape(16, 1), name="iotaq")
    # gather source view: [2048 rows, 4096] — row = vol*256 + dd*4 + hchunk
    xrows = x.ap().rearrange("v p (a b) -> (v p a) b", a=2)

    with TileContext(nc) as tc:
        with (
            tc.tile_pool(name="xc", bufs=3) as xpool,
            tc.tile_pool(name="pw", bufs=3) as ppool,
            tc.tile_pool(name="sm", bufs=2) as spool,
            tc.tile_pool(name="ob", bufs=2) as opool,
            tc.tile_pool(name="big", bufs=1) as bpool,
        ):
          def body(_iv=None):
            base = bpool.tile([128, 1], F32, tag="base")
            nc.sync.dma_start(base[:, :], iota_base_c.ap()[:, :])
            iotaq = bpool.tile([16, 1], I32, tag="iotaq")
            nc.sync.dma_start(iotaq[:, :], iotaq_c.ap()[:, :])
            scal = bpool.tile([1, 64], I32, tag="scal")

            def ts(dst, src, s1, op0):
                return nc.vector.tensor_scalar(
                    out=dst, in0=src, scalar1=s1, scalar2=None, op0=op0
                )

            for s in range(S_PER_CORE):
                power = ppool.tile([128, FREE], F32, tag="pw")
                # stream both channels, build power map
                for k in range(FREE // CHUNK):
                    sl = slice(k * CHUNK, (k + 1) * CHUNK)
                    x0 = xpool.tile([128, CHUNK], F32, tag="xc")
                    nc.sync.dma_start(x0[:, :], x[2 * s, :, sl])
                    nc.scalar.square(power[:, sl], x0[:, :])
                    x1 = xpool.tile([128, CHUNK], F32, tag="xc")
                    nc.sync.dma_start(x1[:, :], x[2 * s + 1, :, sl])
                    nc.scalar.square(x1[:, :], x1[:, :])  # in place
                    nc.vector.tensor_add(power[:, sl], power[:, sl], x1[:, :])

                # per-partition top-1 value + index
                max8 = spool.tile([128, 8], F32, tag="max8")
                idx8 = spool.tile([128, 8], mybir.dt.uint32, tag="idx8")
                nc.vector.max(out=max8[:, :], in_=power[:, :])
                nc.vector.max_index(out=idx8[:, :], in_max=max8[:, :], in_values=power[:, :])

                # global argmax with lowest-flat tie-break
                flatf = spool.tile([128, 1], F32, tag="flatf")
                nc.vector.tensor_copy(flatf[:, :], idx8[:, 0:1])      # uint32 -> f32
                nc.vector.tensor_add(flatf[:, :], flatf[:, :], base[:, :])
                allmax = spool.tile([128, 1], F32, tag="allmax")
                nc.gpsimd.partition_all_reduce(allmax[:, :], max8[:, 0:1], 128, ReduceOp.max)
                eq = spool.tile([128, 1], F32, tag="eq")
                nc.vector.tensor_tensor(out=eq[:, :], in0=max8[:, 0:1], in1=allmax[:, :], op=A.is_equal)
                candneg = spool.tile([128, 1], F32, tag="candneg")
                nc.vector.scalar_tensor_tensor(
                    out=candneg[:, :], in0=eq[:, :], scalar=BIG, in1=flatf[:, :],
                    op0=A.mult, op1=A.subtract,
                )
                allcand = spool.tile([128, 1], F32, tag="allcand")
                nc.gpsimd.partition_all_reduce(allcand[:, :], candneg[:, :], 128, ReduceOp.max)

                # flat = BIG - allcand -> int32 scalar column block for sample s
                def C(j):
                    return scal[:, 16 * s + j : 16 * s + j + 1]

                flat32 = spool.tile([1, 1], F32, tag="flat32")
                nc.vector.tensor_scalar(
                    out=flat32[:, :], in0=allcand[0:1, 0:1], scalar1=BIG, scalar2=-1.0,
                    op0=A.subtract, op1=A.mult,
                )
                nc.vector.tensor_copy(C(0), flat32[:, :])             # f32 -> int32
                ts(C(1), C(0), 14, A.logical_shift_right)             # d
                ts(C(2), C(0), 7, A.logical_shift_right)
                ts(C(2), C(2), 127, A.bitwise_and)                    # h
                ts(C(3), C(0), 127, A.bitwise_and)                    # w
                ts(C(4), C(2), 112, A.add)
                ts(C(4), C(4), 127, A.bitwise_and)                    # h0
                ts(C(5), C(3), 112, A.add)
                w_w0 = ts(C(5), C(5), 127, A.bitwise_and)             # w0
                w_sh = ts(C(6), C(4), 31, A.bitwise_and)              # s_h
                ts(C(9), C(1), 56, A.add)                             # d + 56
                ts(C(10), C(4), 5, A.logical_shift_right)             # c0
                ts(C(11), C(4), 31, A.add)
                ts(C(11), C(11), 127, A.bitwise_and)
                ts(C(11), C(11), 5, A.logical_shift_right)            # c1

                # gather row indices for this sample: 64 idxs in wrapped
                # [16, 4] int16 layout; position n = chunk*32 + c*16 + i
                bc3 = spool.tile([16, 3], I32, tag="bc3")
                nc.gpsimd.partition_broadcast(bc3[:, :], scal[0:1, 16 * s + 9 : 16 * s + 12], channels=16)
                dterm = spool.tile([16, 1], I32, tag="dterm")
                nc.vector.tensor_tensor(out=dterm[:, :], in0=iotaq[:, :], in1=bc3[:, 0:1], op=A.add)
                ts(dterm[:, :], dterm[:, :], 63, A.bitwise_and)
                ts(dterm[:, :], dterm[:, :], 2, A.logical_shift_left)
                idx32 = spool.tile([16, 6], I32, tag="idx32")
                nc.vector.tensor_tensor(out=idx32[:, 4:5], in0=dterm[:, :], in1=bc3[:, 1:2], op=A.add)
                nc.vector.tensor_tensor(out=idx32[:, 5:6], in0=dterm[:, :], in1=bc3[:, 2:3], op=A.add)
                for t in range(4):
                    ts(idx32[:, t : t + 1], idx32[:, 4 + t // 2 : 5 + t // 2], (2 * s + t % 2) * 256, A.add)
                idx16 = spool.tile([16, 4], I16, tag="idx16")
                nc.vector.tensor_copy(idx16[:, :], idx32[:, 0:4])
                idxrep = spool.tile([128, 4], I16, tag="idxrep")
                for g in range(8):
                    nc.sync.dma_start(idxrep[16 * g : 16 * g + 16, :], idx16[:, :])

                # 64 x 16KB gather: all window rows for this sample
                G = ppool.tile([128, 4096], F32, tag="pw")
                nc.gpsimd.dma_gather(
                    out_ap=G[:, :].rearrange("p (a b) -> p a b", a=1),
                    in_ap=xrows,
                    idxs_ap=idxrep[:, :],
                    num_idxs=64,
                    num_idxs_reg=64,
                    elem_size=4096,
                )

                # merge chunk halves into partition-base-0 doubled tile
                Ds = ppool.tile([32, 10240], F32, tag="pw")
                d3 = Ds[:, :].rearrange("p (a b) -> p a b", b=160)
                g3a = G[0:32, :].rearrange("p (a b) -> p a b", b=128)
                g3b = G[32:64, :].rearrange("p (a b) -> p a b", b=128)
                nc.scalar.copy(d3[:, 0:32, 0:128], g3a[:, :, :])
                nc.vector.tensor_copy(d3[:, 32:64, 0:128], g3b[:, :, :])
                nc.scalar.copy(d3[:, 0:32, 128:160], g3a[:, :, 0:32])
                nc.vector.tensor_copy(d3[:, 32:64, 128:160], g3b[:, :, 0:32])

                # dynamic window selection
                li_sh, (sh_val,) = nc.values_load_multi_w_load_instructions(
                    scal[0:1, 16 * s + 6 : 16 * s + 7], engines=(DVE,),
                    min_val=0, max_val=32, skip_runtime_bounds_check=True,
                )
                li_w0, (w0_val,) = nc.values_load_multi_w_load_instructions(
                    scal[0:1, 16 * s + 5 : 16 * s + 6], engines=(DVE,),
                    min_val=0, max_val=128, skip_runtime_bounds_check=True,
                )
                for L in li_sh:
                    add_dep_helper(L.ins, w_sh.ins, sync=True, reason="reg load after s_h write")
                for L in li_w0:
                    add_dep_helper(L.ins, w_w0.ins, sync=True, reason="reg load after w0 write")
                out_sb = opool.tile([32, 1024], F32, tag="out_sb")
                o3 = out_sb[:, :].rearrange("p (a b) -> p a b", a=32)
                sel = d3[0:32, bass.ds(sh_val, 32), bass.ds(w0_val, 32)]
                nc.vector.tensor_copy(o3[:, :, :], sel)
                nc.sync.dma_start(y[32 * s : 32 * s + 32, :], out_sb[:, :])

          if loop_k is None:
              body()
          else:
              with tc.For_i(0, loop_k, 1) as iv:
                  body(iv)

    nc.compile()
    return nc


def get_nc(loop_k=None):
    key = ("nc", loop_k)
    if key not in _cache:
        _cache[key] = _build(loop_k)
    return _cache[key]


def kernel(x: np.ndarray, **run_kwargs) -> np.ndarray:
    assert x.shape == (32, 2, 64, 128, 128) and x.dtype == np.float32
    nc = get_nc()
    in_maps = []
    for c in range(N_CORES):
        xc = x[c * S_PER_CORE : (c + 1) * S_PER_CORE]           # [4, 2, 64, 128, 128]
        xc = np.ascontiguousarray(xc).reshape(N_VOLS, 128, FREE)
        in_maps.append({"x": xc})
    res = run_bass_kernel_spmd(nc, in_maps, core_ids=list(range(N_CORES)), **run_kwargs)
    out = np.empty((32, 2, 16, 32, 32), dtype=np.float32)
    for c in range(N_CORES):
        yc = res.results[c]["y"].reshape(S_PER_CORE, 2, 16, 32, 32)
        out[c * S_PER_CORE : (c + 1) * S_PER_CORE] = yc
    if run_kwargs:
        return out, res
    return out



# revision 6
# speedup vs baseline: 1.3809x; 1.3809x over previous
"""CenterCut2 Trainium2 kernel (v2 — pipelined).

For each sample b: find argmax of power = sum_c x[b,c]^2 over the (D,H,W)
volume, then extract the 16x32x32 window centered on the peak with circular
wraparound (equivalent to reference's per-sample roll + center crop).

Sharding: pure data parallelism, 4 samples per core across 8 cores.

v2 structure (per core, samples s=0..3, volumes laid out [128, 8192] with
flat voxel index = p*8192 + f, f = (h%64)*128 + w, p = d*2 + (h>=64)):
  - Stream 4 chunks of [128, 4096] per sample (x0c0, x1c0, x0c1, x1c1).
  - ACT squares x0 into the power tile and x1 in place; DVE adds.
  - DVE max8 over each 1024-column group -> gmax[128, 64] (8 groups x 8).
  - Tail A: max8+max_index over gmax give per-partition (max, group);
    three gpsimd partition_all_reduce max ops with a BIG-constant
    tie-break select the global winner exactly (lowest flat index on
    ties): (1) global max V, (2) winner prefix p*8192+g*1024, (3) the
    in-group index j from a single 1024-wide max_index on the winning
    group (dynamic ds offset). flat = prefix + j; DVE decodes d/h/w and
    builds gather row indices directly on all 128 partitions (iota p%16,
    one partition_broadcast — no idx replication DMAs).
  - Two 32-descriptor dma_gathers (16KB each) land the 64 window h-rows
    on partitions 0..31 as [32, 64h, 128w]; a w-doubled [32, 32, 160]
    tile (ds(sh) row select) and one ds(w0) strided copy produce the
    [32, 1024] output; out DMA via the scalar-engine HWDGE ring so the
    sync ring only carries stream DMAs.
  - Emission is software-pipelined: stream DMAs for sample s+1 are
    emitted before sample s's tail so the sync ring never head-of-line
    blocks the stream.
"""
import sys

sys.path.insert(0, "/opt/trn_rl_repo")

import numpy as np

import concourse.bass as bass
import concourse.bacc as bacc
import concourse.mybir as mybir
from concourse.tile import TileContext
from concourse.tile_rust import add_dep_helper
from concourse.bass_utils import run_bass_kernel_spmd
from concourse.bass_isa import ReduceOp

F32 = mybir.dt.float32
I32 = mybir.dt.int32
I16 = mybir.dt.int16
U32 = mybir.dt.uint32
A = mybir.AluOpType
DVE = mybir.EngineType.DVE
ACTE = mybir.EngineType.Activation

N_CORES = 8
S_PER_CORE = 4          # samples per core
N_VOLS = 2 * S_PER_CORE # channel volumes per core
VOL = 64 * 128 * 128    # voxels per volume
FREE = VOL // 128       # 8192 free elements per partition
BIG = float(1 << 21)

_cache = {}


def _build():
    nc = bacc.Bacc("TRN2", target_bir_lowering=False, debug=False, num_devices=N_CORES)
    x = nc.dram_tensor("x", [N_VOLS, 128, FREE], F32, kind="ExternalInput")
    y = nc.dram_tensor("y", [128, 1024], F32, kind="ExternalOutput")

    base_c = nc.inline_tensor(
        (np.arange(128, dtype=np.float32) * FREE).reshape(128, 1), name="base8192"
    )
    iota16_c = nc.inline_tensor(
        (np.arange(128, dtype=np.int32) % 16).reshape(128, 1), name="iota16"
    )
    # gather source view: [2048 rows, 4096] — row = vol*256 + dd*4 + hchunk
    xrows = x.ap().rearrange("v p (a b) -> (v p a) b", a=2)

    with TileContext(nc) as tc:
        with (
            tc.tile_pool(name="xc", bufs=4) as xpool,
            tc.tile_pool(name="pw", bufs=2) as ppool,
            tc.tile_pool(name="gm", bufs=2) as gpool,
            tc.tile_pool(name="sm", bufs=2) as spool,
            tc.tile_pool(name="tg", bufs=1) as tpool,
            tc.tile_pool(name="dsl", bufs=1) as dpool,
            tc.tile_pool(name="ob", bufs=2) as opool,
            tc.tile_pool(name="big", bufs=1) as bpool,
        ):
            base = bpool.tile([128, 1], F32, tag="base")
            nc.sync.dma_start(base[:, :], base_c.ap()[:, :])
            iota16 = bpool.tile([128, 1], I32, tag="iota16")
            nc.sync.dma_start(iota16[:, :], iota16_c.ap()[:, :])
            scal = bpool.tile([1, 64], I32, tag="scal")

            def ts(dst, src, s1, op0, s2=None, op1=None):
                kw = {}
                if op1 is not None:
                    kw["op1"] = op1
                return nc.vector.tensor_scalar(
                    out=dst, in0=src, scalar1=s1, scalar2=s2, op0=op0, **kw
                )

            st = {}

            def stream(s):
                xcs = []
                for k in range(2):
                    sl = slice(k * 4096, (k + 1) * 4096)
                    x0 = xpool.tile([128, 4096], F32, tag="xc")
                    nc.sync.dma_start(x0[:, :], x[2 * s, :, sl])
                    x1 = xpool.tile([128, 4096], F32, tag="xc")
                    nc.sync.dma_start(x1[:, :], x[2 * s + 1, :, sl])
                    xcs.append((x0, x1))
                power = ppool.tile([128, FREE], F32, tag="pw")
                gmax = gpool.tile([128, 64], F32, tag="gm")
                st[s] = dict(xcs=xcs, power=power, gmax=gmax)

            def compute_chunk(s, k):
                d = st[s]
                x0, x1 = d["xcs"][k]
                power, gmax = d["power"], d["gmax"]
                sl = slice(k * 4096, (k + 1) * 4096)
                nc.scalar.square(power[:, sl], x0[:, :])
                nc.scalar.square(x1[:, :], x1[:, :])  # in place
                nc.vector.tensor_add(power[:, sl], power[:, sl], x1[:, :])
                for g in range(4):
                    gi = 4 * k + g
                    nc.vector.max(
                        out=gmax[:, 8 * gi : 8 * gi + 8],
                        in_=power[:, 1024 * gi : 1024 * gi + 1024],
                    )

            def tail_a(s):
                d = st[s]
                power, gmax = d["power"], d["gmax"]
                b = 16 * s

                def C(j):
                    return scal[:, b + j : b + j + 1]

                # per-partition best value + best group (lowest g on ties)
                pmax8 = spool.tile([128, 8], F32, tag="pmax8")
                nc.vector.max(out=pmax8[:, :], in_=gmax[:, :])
                gcol8 = spool.tile([128, 8], U32, tag="gcol8")
                nc.vector.max_index(out=gcol8[:, :], in_max=pmax8[:, :], in_values=gmax[:, :])
                # global max V
                allmax = spool.tile([128, 1], F32, tag="allmax")
                nc.gpsimd.partition_all_reduce(allmax[:, :], pmax8[:, 0:1], 128, ReduceOp.max)
                eq = spool.tile([128, 1], F32, tag="eq")
                nc.vector.tensor_tensor(out=eq[:, :], in0=pmax8[:, 0:1], in1=allmax[:, :], op=A.is_equal)
                # winner prefix = p*8192 + g*1024 (lowest on ties)
                gi32 = spool.tile([128, 1], I32, tag="gi32")
                nc.vector.tensor_copy(gi32[:, :], gcol8[:, 0:1])
                ts(gi32[:, :], gi32[:, :], 3, A.logical_shift_right)
                ts(gi32[:, :], gi32[:, :], 10, A.logical_shift_left)
                gpf = spool.tile([128, 1], F32, tag="gpf")
                nc.vector.tensor_copy(gpf[:, :], gi32[:, :])
                prefixf = spool.tile([128, 1], F32, tag="prefixf")
                nc.vector.tensor_tensor(out=prefixf[:, :], in0=gpf[:, :], in1=base[:, :], op=A.add)
                cn2 = spool.tile([128, 1], F32, tag="cn2")
                nc.vector.scalar_tensor_tensor(
                    out=cn2[:, :], in0=eq[:, :], scalar=BIG, in1=prefixf[:, :],
                    op0=A.mult, op1=A.subtract,
                )
                ac2 = spool.tile([128, 1], F32, tag="ac2")
                nc.gpsimd.partition_all_reduce(ac2[:, :], cn2[:, :], 128, ReduceOp.max)
                prefF = spool.tile([1, 1], F32, tag="prefF")
                nc.vector.tensor_scalar(
                    out=prefF[:, :], in0=ac2[0:1, 0:1], scalar1=BIG, scalar2=-1.0,
                    op0=A.subtract, op1=A.mult,
                )
                nc.vector.tensor_copy(C(10), prefF[:, :])          # prefix_i
                w_go = ts(C(11), C(10), 8191, A.bitwise_and)       # goff = g*1024
                li_go, (go_val,) = nc.values_load_multi_w_load_instructions(
                    C(11), engines=(DVE,), min_val=0, max_val=FREE - 1024,
                    skip_runtime_bounds_check=True,
                )
                for L in li_go:
                    add_dep_helper(L.ins, w_go.ins, sync=True, reason="goff reg")
                # in-group index of V on the winning partition
                j8 = spool.tile([128, 8], U32, tag="j8")
                nc.vector.max_index(
                    out=j8[:, :], in_max=pmax8[:, :],
                    in_values=power[:, bass.ds(go_val, 1024)],
                )
                jf = spool.tile([128, 1], F32, tag="jf")
                nc.vector.tensor_copy(jf[:, :], j8[:, 0:1])
                cn3 = spool.tile([128, 1], F32, tag="cn3")
                nc.vector.scalar_tensor_tensor(
                    out=cn3[:, :], in0=eq[:, :], scalar=BIG, in1=jf[:, :],
                    op0=A.mult, op1=A.subtract,
                )
                ac3 = spool.tile([128, 1], F32, tag="ac3")
                nc.gpsimd.partition_all_reduce(ac3[:, :], cn3[:, :], 128, ReduceOp.max)
                jF = spool.tile([1, 1], F32, tag="jF")
                nc.vector.tensor_scalar(
                    out=jF[:, :], in0=ac3[0:1, 0:1], scalar1=BIG, scalar2=-1.0,
                    op0=A.subtract, op1=A.mult,
                )
                nc.vector.tensor_copy(C(12), jF[:, :])
                nc.vector.tensor_tensor(out=C(0), in0=C(10), in1=C(12), op=A.add)  # flat
                # decode
                ts(C(1), C(0), 14, A.logical_shift_right)              # d
                ts(C(2), C(0), 7, A.logical_shift_right)
                ts(C(2), C(2), 127, A.bitwise_and)                     # h
                ts(C(3), C(0), 127, A.bitwise_and)                     # w
                ts(C(4), C(2), 112, A.add)
                ts(C(4), C(4), 127, A.bitwise_and)                     # h0
                ts(C(5), C(3), 112, A.add)
                w_w0 = ts(C(5), C(5), 127, A.bitwise_and)              # w0
                w_sh = ts(C(6), C(4), 31, A.bitwise_and)               # sh
                ts(C(7), C(1), 56, A.add)                              # d + 56
                ts(C(8), C(4), 5, A.logical_shift_right)               # c0
                ts(C(13), C(4), 31, A.add)
                ts(C(13), C(13), 127, A.bitwise_and)
                ts(C(9), C(13), 5, A.logical_shift_right)              # c1
                # gather row indices on all 128 partitions (8 replicated
                # 16-partition groups via iota p%16)
                bc = spool.tile([128, 3], I32, tag="bc")
                nc.gpsimd.partition_broadcast(bc[:, :], scal[0:1, b + 7 : b + 10], channels=128)
                dterm = spool.tile([128, 1], I32, tag="dterm")
                nc.vector.tensor_tensor(out=dterm[:, :], in0=iota16[:, :], in1=bc[:, 0:1], op=A.add)
                ts(dterm[:, :], dterm[:, :], 63, A.bitwise_and)
                ts(dterm[:, :], dterm[:, :], 2, A.logical_shift_left)
                idx32 = spool.tile([128, 4], I32, tag="idx32")
                nc.vector.tensor_tensor(out=idx32[:, 0:1], in0=dterm[:, :], in1=bc[:, 1:2], op=A.add)
                nc.vector.tensor_tensor(out=idx32[:, 2:3], in0=dterm[:, :], in1=bc[:, 2:3], op=A.add)
                ts(idx32[:, 0:1], idx32[:, 0:1], 512 * s, A.add)
                ts(idx32[:, 1:2], idx32[:, 0:1], 256, A.add)
                ts(idx32[:, 2:3], idx32[:, 2:3], 512 * s, A.add)
                ts(idx32[:, 3:4], idx32[:, 2:3], 256, A.add)
                idx16 = spool.tile([128, 4], I16, tag="idx16")
                nc.vector.tensor_copy(idx16[:, :], idx32[:, :])
                # two 32-descriptor gathers: chunk c0 rows then chunk c1 rows,
                # both landing on partitions 0..31 (p = c*16 + i)
                T = tpool.tile([128, 8192], F32, tag="tg")
                nc.gpsimd.dma_gather(
                    out_ap=T[:, 0:4096].rearrange("p (a b) -> p a b", a=1),
                    in_ap=xrows, idxs_ap=idx16[:, 0:2],
                    num_idxs=32, num_idxs_reg=32, elem_size=4096,
                )
                nc.gpsimd.dma_gather(
                    out_ap=T[:, 4096:8192].rearrange("p (a b) -> p a b", a=1),
                    in_ap=xrows, idxs_ap=idx16[:, 2:4],
                    num_idxs=32, num_idxs_reg=32, elem_size=4096,
                )
                d["T"] = T
                d["w_sh"] = w_sh
                d["w_w0"] = w_w0

            def tail_b(s):
                d = st[s]
                T = d["T"]
                T3v = T[0:32, :].rearrange("p (h w) -> p h w", w=128)
                b = 16 * s
                li_sh, (sh_d,) = nc.values_load_multi_w_load_instructions(
                    scal[0:1, b + 6 : b + 7], engines=(DVE,), min_val=0, max_val=32,
                    skip_runtime_bounds_check=True,
                )
                for L in li_sh:
                    add_dep_helper(L.ins, d["w_sh"].ins, sync=True, reason="sh reg dve")
                li_shA, (sh_a,) = nc.values_load_multi_w_load_instructions(
                    scal[0:1, b + 6 : b + 7], engines=(ACTE,), min_val=0, max_val=32,
                    skip_runtime_bounds_check=True,
                )
                for L in li_shA:
                    add_dep_helper(L.ins, d["w_sh"].ins, sync=True, reason="sh reg act")
                li_w0, (w0_d,) = nc.values_load_multi_w_load_instructions(
                    scal[0:1, b + 5 : b + 6], engines=(DVE,), min_val=0, max_val=128,
                    skip_runtime_bounds_check=True,
                )
                for L in li_w0:
                    add_dep_helper(L.ins, d["w_w0"].ins, sync=True, reason="w0 reg")
                # w-doubled window rows [32, 32h, 160w]; ds(sh) row select
                Dsel = dpool.tile([32, 5120], F32, tag="dsl")
                D3 = Dsel[:, :].rearrange("p (h w) -> p h w", w=160)
                nc.scalar.copy(D3[:, :, 0:128], T3v[:, bass.ds(sh_a, 32), :])
                nc.vector.tensor_copy(D3[:, :, 128:160], T3v[:, bass.ds(sh_d, 32), 0:32])
                out_sb = opool.tile([32, 1024], F32, tag="ob")
                o3 = out_sb[:, :].rearrange("p (a b) -> p a b", a=32)
                nc.vector.tensor_copy(o3[:, :, :], D3[:, :, bass.ds(w0_d, 32)])
                nc.scalar.dma_start(y[32 * s : 32 * s + 32, :], out_sb[:, :])

            for s in range(S_PER_CORE):
                stream(s)
                if s >= 1:
                    tail_a(s - 1)
                compute_chunk(s, 0)
                if s >= 1:
                    tail_b(s - 1)
                compute_chunk(s, 1)
            tail_a(S_PER_CORE - 1)
            tail_b(S_PER_CORE - 1)

    nc.compile()
    return nc


def get_nc():
    key = ("nc",)
    if key not in _cache:
        _cache[key] = _build()
    return _cache[key]


def kernel(x: np.ndarray, **run_kwargs) -> np.ndarray:
    assert x.shape == (32, 2, 64, 128, 128) and x.dtype == np.float32
    nc = get_nc()
    in_maps = []
    for c in range(N_CORES):
        xc = x[c * S_PER_CORE : (c + 1) * S_PER_CORE]           # [4, 2, 64, 128, 128]
        xc = np.ascontiguousarray(xc).reshape(N_VOLS, 128, FREE)
        in_maps.append({"x": xc})
    res = run_bass_kernel_spmd(nc, in_maps, core_ids=list(range(N_CORES)), **run_kwargs)
    out = np.empty((32, 2, 16, 32, 32), dtype=np.float32)
    for c in range(N_CORES):
        yc = res.results[c]["y"].reshape(S_PER_CORE, 2, 16, 32, 32)
        out[c * S_PER_CORE : (c + 1) * S_PER_CORE] = yc
    if run_kwargs:
        return out, res
    return out


# revision 11
# speedup vs baseline: 1.5385x; 1.1141x over previous
"""CenterCut2 Trainium2 kernel (v2 — pipelined).

For each sample b: find argmax of power = sum_c x[b,c]^2 over the (D,H,W)
volume, then extract the 16x32x32 window centered on the peak with circular
wraparound (equivalent to reference's per-sample roll + center crop).

Sharding: pure data parallelism, 4 samples per core across 8 cores.

v2 structure (per core, samples s=0..3, volumes laid out [128, 8192] with
flat voxel index = p*8192 + f, f = (h%64)*128 + w, p = d*2 + (h>=64)):
  - Stream 4 chunks of [128, 4096] per sample (x0c0, x1c0, x0c1, x1c1).
  - ACT squares x0 into the power tile and x1 in place; DVE adds.
  - DVE max8 over each 1024-column group -> gmax[128, 64] (8 groups x 8).
  - Tail A: max8+max_index over gmax give per-partition (max, group);
    three gpsimd partition_all_reduce max ops with a BIG-constant
    tie-break select the global winner exactly (lowest flat index on
    ties): (1) global max V, (2) winner prefix p*8192+g*1024, (3) the
    in-group index j from a single 1024-wide max_index on the winning
    group (dynamic ds offset). flat = prefix + j; DVE decodes d/h/w and
    builds gather row indices directly on all 128 partitions (iota p%16,
    one partition_broadcast — no idx replication DMAs).
  - Two 32-descriptor dma_gathers (16KB each) land the 64 window h-rows
    on partitions 0..31 as [32, 64h, 128w]; a w-doubled [32, 32, 160]
    tile (ds(sh) row select) and one ds(w0) strided copy produce the
    [32, 1024] output; out DMA via the scalar-engine HWDGE ring so the
    sync ring only carries stream DMAs.
  - Emission is software-pipelined: stream DMAs for sample s+1 are
    emitted before sample s's tail so the sync ring never head-of-line
    blocks the stream.
"""
import sys

sys.path.insert(0, "/opt/trn_rl_repo")

import numpy as np

import concourse.bass as bass
import concourse.bacc as bacc
import concourse.mybir as mybir
from concourse.tile import TileContext
from concourse.tile_rust import add_dep_helper
from concourse.bass_utils import run_bass_kernel_spmd
from concourse.bass_isa import ReduceOp

F32 = mybir.dt.float32
I32 = mybir.dt.int32
I16 = mybir.dt.int16
U32 = mybir.dt.uint32
A = mybir.AluOpType
DVE = mybir.EngineType.DVE
ACTE = mybir.EngineType.Activation

N_CORES = 8
S_PER_CORE = 4          # samples per core
N_VOLS = 2 * S_PER_CORE # channel volumes per core
VOL = 64 * 128 * 128    # voxels per volume
FREE = VOL // 128       # 8192 free elements per partition
BIG = float(1 << 21)

_cache = {}


def _build():
    nc = bacc.Bacc("TRN2", target_bir_lowering=False, debug=False, num_devices=N_CORES)
    x = nc.dram_tensor("x", [N_VOLS, 128, FREE], F32, kind="ExternalInput")
    y = nc.dram_tensor("y", [128, 1024], F32, kind="ExternalOutput")

    base_c = nc.inline_tensor(
        (np.arange(128, dtype=np.float32) * FREE).reshape(128, 1), name="base8192"
    )
    iota16_c = nc.inline_tensor(
        (np.arange(128, dtype=np.int32) % 16).reshape(128, 1), name="iota16"
    )
    # gather source view: [2048 rows, 4096] — row = vol*256 + dd*4 + hchunk
    xrows = x.ap().rearrange("v p (a b) -> (v p a) b", a=2)

    with TileContext(nc) as tc:
        with (
            tc.tile_pool(name="xc", bufs=10) as xpool,
            tc.tile_pool(name="pw", bufs=2) as ppool,
            tc.tile_pool(name="gm", bufs=2) as gpool,
            tc.tile_pool(name="sm", bufs=2) as spool,
            tc.tile_pool(name="dsl", bufs=1) as dpool,
            tc.tile_pool(name="ob", bufs=2) as opool,
            tc.tile_pool(name="big", bufs=1) as bpool,
        ):
            base = bpool.tile([128, 1], F32, tag="base")
            nc.sync.dma_start(base[:, :], base_c.ap()[:, :])
            iota16 = bpool.tile([128, 1], I32, tag="iota16")
            nc.sync.dma_start(iota16[:, :], iota16_c.ap()[:, :])
            scal = bpool.tile([1, 64], I32, tag="scal")

            def ts(dst, src, s1, op0, s2=None, op1=None):
                kw = {}
                if op1 is not None:
                    kw["op1"] = op1
                return nc.vector.tensor_scalar(
                    out=dst, in0=src, scalar1=s1, scalar2=s2, op0=op0, **kw
                )

            st = {}

            def stream(s):
                xcs = []
                for k in range(4):
                    sl = slice(k * 2048, (k + 1) * 2048)
                    x0 = xpool.tile([128, 2048], F32, tag="xc")
                    nc.sync.dma_start(x0[:, :], x[2 * s, :, sl])
                    x1 = xpool.tile([128, 2048], F32, tag="xc")
                    nc.sync.dma_start(x1[:, :], x[2 * s + 1, :, sl])
                    xcs.append((x0, x1))
                power = ppool.tile([128, FREE], F32, tag="pw")
                gmax = gpool.tile([128, 64], F32, tag="gm")
                st[s] = dict(xcs=xcs, power=power, gmax=gmax)

            def compute_chunk(s, k):
                d = st[s]
                x0, x1 = d["xcs"][k]
                power, gmax = d["power"], d["gmax"]
                sl = slice(k * 2048, (k + 1) * 2048)
                nc.scalar.square(power[:, sl], x0[:, :])
                nc.scalar.square(x1[:, :], x1[:, :])  # in place
                nc.vector.tensor_add(power[:, sl], power[:, sl], x1[:, :])
                for g in range(2):
                    gi = 2 * k + g
                    nc.vector.max(
                        out=gmax[:, 8 * gi : 8 * gi + 8],
                        in_=power[:, 1024 * gi : 1024 * gi + 1024],
                    )

            def tail_a(s):
                d = st[s]
                power, gmax = d["power"], d["gmax"]
                b = 16 * s

                def C(j):
                    return scal[:, b + j : b + j + 1]

                # per-partition best value + best group (lowest g on ties)
                pmax8 = spool.tile([128, 8], F32, tag="pmax8")
                nc.vector.max(out=pmax8[:, :], in_=gmax[:, :])
                gcol8 = spool.tile([128, 8], U32, tag="gcol8")
                nc.vector.max_index(out=gcol8[:, :], in_max=pmax8[:, :], in_values=gmax[:, :])
                # global max V
                allmax = spool.tile([128, 1], F32, tag="allmax")
                nc.gpsimd.partition_all_reduce(allmax[:, :], pmax8[:, 0:1], 128, ReduceOp.max)
                eq = spool.tile([128, 1], F32, tag="eq")
                nc.vector.tensor_tensor(out=eq[:, :], in0=pmax8[:, 0:1], in1=allmax[:, :], op=A.is_equal)
                # winner prefix = p*8192 + g*1024 (lowest on ties)
                gi32 = spool.tile([128, 1], I32, tag="gi32")
                nc.vector.tensor_copy(gi32[:, :], gcol8[:, 0:1])
                ts(gi32[:, :], gi32[:, :], 3, A.logical_shift_right)
                ts(gi32[:, :], gi32[:, :], 10, A.logical_shift_left)
                gpf = spool.tile([128, 1], F32, tag="gpf")
                nc.vector.tensor_copy(gpf[:, :], gi32[:, :])
                prefixf = spool.tile([128, 1], F32, tag="prefixf")
                nc.vector.tensor_tensor(out=prefixf[:, :], in0=gpf[:, :], in1=base[:, :], op=A.add)
                cn2 = spool.tile([128, 1], F32, tag="cn2")
                nc.vector.scalar_tensor_tensor(
                    out=cn2[:, :], in0=eq[:, :], scalar=BIG, in1=prefixf[:, :],
                    op0=A.mult, op1=A.subtract,
                )
                ac2 = spool.tile([128, 1], F32, tag="ac2")
                nc.gpsimd.partition_all_reduce(ac2[:, :], cn2[:, :], 128, ReduceOp.max)
                prefF = spool.tile([1, 1], F32, tag="prefF")
                nc.vector.tensor_scalar(
                    out=prefF[:, :], in0=ac2[0:1, 0:1], scalar1=BIG, scalar2=-1.0,
                    op0=A.subtract, op1=A.mult,
                )
                nc.vector.tensor_copy(C(10), prefF[:, :])          # prefix_i
                w_go = ts(C(11), C(10), 8191, A.bitwise_and)       # goff = g*1024
                li_go, (go_val,) = nc.values_load_multi_w_load_instructions(
                    C(11), engines=(DVE,), min_val=0, max_val=FREE - 1024,
                    skip_runtime_bounds_check=True,
                )
                for L in li_go:
                    add_dep_helper(L.ins, w_go.ins, sync=True, reason="goff reg")
                # in-group index of V on the winning partition
                j8 = spool.tile([128, 8], U32, tag="j8")
                nc.vector.max_index(
                    out=j8[:, :], in_max=pmax8[:, :],
                    in_values=power[:, bass.ds(go_val, 1024)],
                )
                jf = spool.tile([128, 1], F32, tag="jf")
                nc.vector.tensor_copy(jf[:, :], j8[:, 0:1])
                cn3 = spool.tile([128, 1], F32, tag="cn3")
                nc.vector.scalar_tensor_tensor(
                    out=cn3[:, :], in0=eq[:, :], scalar=BIG, in1=jf[:, :],
                    op0=A.mult, op1=A.subtract,
                )
                ac3 = spool.tile([128, 1], F32, tag="ac3")
                nc.gpsimd.partition_all_reduce(ac3[:, :], cn3[:, :], 128, ReduceOp.max)
                jF = spool.tile([1, 1], F32, tag="jF")
                nc.vector.tensor_scalar(
                    out=jF[:, :], in0=ac3[0:1, 0:1], scalar1=BIG, scalar2=-1.0,
                    op0=A.subtract, op1=A.mult,
                )
                nc.vector.tensor_copy(C(12), jF[:, :])
                nc.vector.tensor_tensor(out=C(0), in0=C(10), in1=C(12), op=A.add)  # flat
                # decode
                ts(C(1), C(0), 14, A.logical_shift_right)              # d
                ts(C(2), C(0), 7, A.logical_shift_right)
                ts(C(2), C(2), 127, A.bitwise_and)                     # h
                ts(C(3), C(0), 127, A.bitwise_and)                     # w
                ts(C(4), C(2), 112, A.add)
                ts(C(4), C(4), 127, A.bitwise_and)                     # h0
                ts(C(5), C(3), 112, A.add)
                w_w0 = ts(C(5), C(5), 127, A.bitwise_and)              # w0
                w_sh = ts(C(6), C(4), 31, A.bitwise_and)               # sh
                ts(C(7), C(1), 56, A.add)                              # d + 56
                ts(C(8), C(4), 5, A.logical_shift_right)               # c0
                ts(C(13), C(4), 31, A.add)
                ts(C(13), C(13), 127, A.bitwise_and)
                ts(C(9), C(13), 5, A.logical_shift_right)              # c1
                # gather row indices on all 128 partitions (8 replicated
                # 16-partition groups via iota p%16)
                bc = spool.tile([128, 3], I32, tag="bc")
                nc.gpsimd.partition_broadcast(bc[:, :], scal[0:1, b + 7 : b + 10], channels=128)
                dterm = spool.tile([128, 1], I32, tag="dterm")
                nc.vector.tensor_tensor(out=dterm[:, :], in0=iota16[:, :], in1=bc[:, 0:1], op=A.add)
                ts(dterm[:, :], dterm[:, :], 63, A.bitwise_and)
                ts(dterm[:, :], dterm[:, :], 2, A.logical_shift_left)
                idx32 = spool.tile([128, 4], I32, tag="idx32")
                nc.vector.tensor_tensor(out=idx32[:, 0:1], in0=dterm[:, :], in1=bc[:, 1:2], op=A.add)
                nc.vector.tensor_tensor(out=idx32[:, 2:3], in0=dterm[:, :], in1=bc[:, 2:3], op=A.add)
                ts(idx32[:, 0:1], idx32[:, 0:1], 512 * s, A.add)
                ts(idx32[:, 1:2], idx32[:, 0:1], 256, A.add)
                ts(idx32[:, 2:3], idx32[:, 2:3], 512 * s, A.add)
                ts(idx32[:, 3:4], idx32[:, 2:3], 256, A.add)
                idx16 = spool.tile([128, 4], I16, tag="idx16")
                nc.vector.tensor_copy(idx16[:, :], idx32[:, :])
                # two 32-descriptor gathers: chunk c0 rows then chunk c1 rows,
                # both landing on partitions 0..31 (p = c*16 + i)
                # reuse the power tile as the gather landing zone — the find
                # above was its last read, so the WAR sync is already implied
                # by the idx dependency chain
                T = power
                nc.gpsimd.dma_gather(
                    out_ap=T[:, 0:4096].rearrange("p (a b) -> p a b", a=1),
                    in_ap=xrows, idxs_ap=idx16[:, 0:2],
                    num_idxs=32, num_idxs_reg=32, elem_size=4096,
                )
                nc.gpsimd.dma_gather(
                    out_ap=T[:, 4096:8192].rearrange("p (a b) -> p a b", a=1),
                    in_ap=xrows, idxs_ap=idx16[:, 2:4],
                    num_idxs=32, num_idxs_reg=32, elem_size=4096,
                )
                d["T"] = T
                d["w_sh"] = w_sh
                d["w_w0"] = w_w0

            def tail_b(s):
                d = st[s]
                T = d["T"]
                T3v = T[0:32, :].rearrange("p (h w) -> p h w", w=128)
                b = 16 * s
                li_sh, (sh_d,) = nc.values_load_multi_w_load_instructions(
                    scal[0:1, b + 6 : b + 7], engines=(DVE,), min_val=0, max_val=32,
                    skip_runtime_bounds_check=True,
                )
                for L in li_sh:
                    add_dep_helper(L.ins, d["w_sh"].ins, sync=True, reason="sh reg dve")
                li_shA, (sh_a,) = nc.values_load_multi_w_load_instructions(
                    scal[0:1, b + 6 : b + 7], engines=(ACTE,), min_val=0, max_val=32,
                    skip_runtime_bounds_check=True,
                )
                for L in li_shA:
                    add_dep_helper(L.ins, d["w_sh"].ins, sync=True, reason="sh reg act")
                li_w0, (w0_d,) = nc.values_load_multi_w_load_instructions(
                    scal[0:1, b + 5 : b + 6], engines=(DVE,), min_val=0, max_val=128,
                    skip_runtime_bounds_check=True,
                )
                for L in li_w0:
                    add_dep_helper(L.ins, d["w_w0"].ins, sync=True, reason="w0 reg")
                # w-doubled window rows [32, 32h, 160w]; ds(sh) row select
                Dsel = dpool.tile([32, 5120], F32, tag="dsl")
                D3 = Dsel[:, :].rearrange("p (h w) -> p h w", w=160)
                nc.scalar.copy(D3[:, :, 0:128], T3v[:, bass.ds(sh_a, 32), :])
                nc.vector.tensor_copy(D3[:, :, 128:160], T3v[:, bass.ds(sh_d, 32), 0:32])
                out_sb = opool.tile([32, 1024], F32, tag="ob")
                o3 = out_sb[:, :].rearrange("p (a b) -> p a b", a=32)
                nc.vector.tensor_copy(o3[:, :, :], D3[:, :, bass.ds(w0_d, 32)])
                nc.scalar.dma_start(y[32 * s : 32 * s + 32, :], out_sb[:, :])

            for s in range(S_PER_CORE):
                stream(s)
                if s >= 1:
                    tail_a(s - 1)
                compute_chunk(s, 0)
                compute_chunk(s, 1)
                if s >= 1:
                    tail_b(s - 1)
                compute_chunk(s, 2)
                compute_chunk(s, 3)
            tail_a(S_PER_CORE - 1)
            tail_b(S_PER_CORE - 1)

    nc.compile()
    return nc


def get_nc():
    key = ("nc",)
    if key not in _cache:
        _cache[key] = _build()
    return _cache[key]


def kernel(x: np.ndarray, **run_kwargs) -> np.ndarray:
    assert x.shape == (32, 2, 64, 128, 128) and x.dtype == np.float32
    nc = get_nc()
    in_maps = []
    for c in range(N_CORES):
        xc = x[c * S_PER_CORE : (c + 1) * S_PER_CORE]           # [4, 2, 64, 128, 128]
        xc = np.ascontiguousarray(xc).reshape(N_VOLS, 128, FREE)
        in_maps.append({"x": xc})
    res = run_bass_kernel_spmd(nc, in_maps, core_ids=list(range(N_CORES)), **run_kwargs)
    out = np.empty((32, 2, 16, 32, 32), dtype=np.float32)
    for c in range(N_CORES):
        yc = res.results[c]["y"].reshape(S_PER_CORE, 2, 16, 32, 32)
        out[c * S_PER_CORE : (c + 1) * S_PER_CORE] = yc
    if run_kwargs:
        return out, res
    return out
